# revision 18
# baseline (speedup 1.0000x reference)
"""Trainium2 Bass kernel for nn_ClosedFormLoss (closed-form matting Laplacian loss).

Math: per (batch, class), y = keep * (vals @ ow) per 3x3 window followed by a
scatter-add, where vals is the 9x9 matting-Laplacian block built from a
per-window 3x3 color covariance inverse.  With S = box3(o), q_ch =
box3(imgn_ch * o), t = q - mu*S, bk = (keep/9 * inv) @ t:
    Lo(p) = o(p)*Nk(p) - A(p) - sum_ch imgn_ch(p) * B_ch(p)
    A  = box3T(keep/9*S - mu.bk),  B_ch = box3T(bk_ch),  Nk = box3T(keep)
and loss_bc = sum_p (Lo + conf*o - tri*conf)^2 / n^2,  n = H*W.

v2 vs baseline: keep9 is folded into the inverse (iv' = keep9*inv) so the
gpsimd muv chain disappears; the ak assembly (keep9*S - sum mu*bk) happens as a
4-field PSUM accumulation inside the A transpose-box; the per-class target is
injected into the same PSUM group via a 100*I @ onehot_c matmul; the color
moment boxes run in bf16 (1 cyc/row); the residual runs in bf16 off partition
slices of the pixel tiles.  Elementwise work is split DVE/Pool/ACT by measured
cost (DVE TT 0.52 ns/col, Pool TT 1.98, Pool copy 1.39, ACT 0.83).

Sharding: 8 cores = 2 batches x 4 row-quarters; each core processes its
quarter as two 64/65-row sub-stripes and emits one partial sum of squares.
"""

import sys
import numpy as np

sys.path.insert(0, "/opt/trn_rl_repo")

from concourse import bacc, mybir, tile  # noqa: E402
from concourse.bass_utils import run_bass_kernel_spmd  # noqa: E402

F32 = mybir.dt.float32
BF16 = mybir.dt.bfloat16
I32 = mybir.dt.int32
OP = mybir.AluOpType
COPY = mybir.ActivationFunctionType.Copy

N_CLASSES = 7
H = W = 513
NWC = 511          # window grid cols
N = H * W
EPS9 = 1e-7 / 9.0
TRI_CONF = 100.0
import ml_dtypes  # noqa: E402
NINE_EFF = float(1.0 / np.float32(np.float32(1.0 / 9.0).astype(ml_dtypes.bfloat16)))

SUB_R0 = [64 * s for s in range(8)]
N_OUT = 65          # rows computed per sub (65 with 1-row overlap)
N_WIN = 67          # window rows per sub: [r0-2, r0+65)
N_PIX = 69          # pixel rows for box inputs: [r0-2, r0+67)
N_M = 71            # trimap rows for dilate: [r0-3, r0+68)
CH0, CH1 = 257, 256  # column chunks for 513-wide transpose-box outputs

_PROGRAM = None


def _build_program():
    nc = bacc.Bacc("TRN2", target_bir_lowering=False, debug=False, num_devices=8)

    cpr_d = nc.declare_dram_parameter("cpr", [2, N_CLASSES, N_PIX, W], F32, isOutput=False)
    img_d = nc.declare_dram_parameter("img", [2, 3, N_PIX, W], F32, isOutput=False)
    trim_d = nc.declare_dram_parameter("trim", [2, N_M, W], I32, isOutput=False)
    band_d = nc.declare_dram_parameter("band", [N_M, N_PIX], F32, isOutput=False)
    vmask_d = nc.declare_dram_parameter("vmask9", [2, N_WIN, 1], F32, isOutput=False)
    rmask_d = nc.declare_dram_parameter("rmask", [2, N_OUT, 1], F32, isOutput=False)
    ones_d = nc.declare_dram_parameter("ones", [N_OUT, 1], F32, isOutput=False)
    ident_d = nc.declare_dram_parameter("ident", [N_OUT, N_OUT], F32, isOutput=False)
    part_d = nc.declare_dram_parameter("partial", [1, 1], F32, isOutput=True)

    with tile.TileContext(nc) as tc:
        with (
            tc.tile_pool(name="sb", bufs=1) as sb,
            tc.tile_pool(name="spc", bufs=4) as spc,
            tc.tile_pool(name="sc2", bufs=3) as sc2,
            tc.tile_pool(name="sc1", bufs=1) as sc1,
            tc.tile_pool(name="cst", bufs=1) as cst,
            tc.tile_pool(name="psf", bufs=5, space="PSUM") as psf,
            tc.tile_pool(name="psb", bufs=3, space="PSUM") as psb,
        ):
            band = cst.tile([N_M, N_PIX], F32, name="band", tag="band")
            band_bf = cst.tile([N_M, N_PIX], BF16, name="band_bf", tag="band_bf")
            ones = cst.tile([N_OUT, 1], F32, name="ones", tag="ones")
            ident_f = cst.tile([N_OUT, N_OUT], F32, name="ident_f", tag="ident_f")
            ident = cst.tile([N_OUT, N_OUT], BF16, name="ident", tag="ident")
            nc.sync.dma_start(band[:], band_d[:])
            nc.sync.dma_start(ones[:], ones_d[:])
            nc.sync.dma_start(ident_f[:], ident_d[:])
            nc.vector.tensor_copy(band_bf[:], band[:])
            nc.vector.tensor_copy(ident[:], ident_f[:])

            def fwd_box(dst_ps, src, wm, nk=N_PIX, nm=N_WIN):
                # dst[m, j] = sum_{dj} sum_k band[k, m] * src[k, j+dj]
                for dj in range(3):
                    nc.tensor.matmul(
                        dst_ps[0:nm, :], wm[0:nk, 0:nm], src[0:nk, dj:dj + NWC],
                        start=(dj == 0), stop=(dj == 2),
                    )

            def t_box(dst_ps, src, c0, c1, wm, start=True, stop=True):
                # transpose box: out pixel col j <- window cols j-2, j-1, j
                first = start
                for djw in range(3):
                    jlo, jhi = max(c0, djw), min(c1, NWC + djw)
                    nc.tensor.matmul(
                        dst_ps[0:N_OUT, jlo - c0:jhi - c0],
                        wm[0:N_WIN, 0:N_OUT],
                        src[0:N_WIN, jlo - djw:jhi - djw],
                        start=first, stop=(stop and djw == 2),
                    )
                    first = False

            def dil_box(dst_ps, src, c0, c1, wm):
                # 3x3 SAME-pad dilate numerator on the pixel grid
                first = True
                for dj in (-1, 0, 1):
                    jlo, jhi = max(c0, -dj), min(c1, W - dj)
                    nc.tensor.matmul(
                        dst_ps[0:N_PIX, jlo - c0:jhi - c0],
                        wm[0:N_M, 0:N_PIX],
                        src[0:N_M, jlo + dj:jhi + dj],
                        start=first, stop=(dj == 1),
                    )
                    first = False

            accm_tot = cst.tile([N_OUT, 1], F32, name="accm_tot", tag="accm_tot")

            for s in range(2):
                # ---- load per-sub inputs ----
                img_raw = [sb.tile([N_PIX, W], F32, name=f"imgr{ch}", tag=f"imgr{ch}") for ch in range(3)]
                for ch in range(3):
                    nc.sync.dma_start(img_raw[ch][:], img_d[s, ch])
                ti_a = sb.tile([N_M, W], I32, name="ti_a", tag="ti_a")
                nc.sync.dma_start(ti_a[:], trim_d[s])
                vmask9 = sb.tile([N_WIN, 1], F32, name="vmask9", tag="vmask9")
                rmask = sb.tile([N_OUT, 1], F32, name="rmask", tag="rmask")
                nc.sync.dma_start(vmask9[:], vmask_d[s])
                nc.sync.dma_start(rmask[:], rmask_d[s])

                # ---- trimap-derived masks (before moments: keep9 feeds iv') ----
                tf_a = sb.tile([N_M, W], BF16, name="tf_a", tag="tf_a")
                nc.vector.tensor_copy(tf_a[:], ti_a[:])        # 0..128 exact in bf16
                ti_o = sb.tile([N_OUT, W], I32, name="ti_o", tag="ti_o")
                nc.sync.dma_start(ti_o[:], trim_d[s, 3:3 + N_OUT])
                tf_o = sb.tile([N_OUT, W], BF16, name="tf_o", tag="tf_o")
                nc.vector.tensor_copy(tf_o[:], ti_o[:])
                m100o = sb.tile([N_OUT, W], BF16, name="m100o", tag="m100o")
                nc.vector.tensor_scalar(m100o[:], tf_o[:], 128.0, TRI_CONF, OP.is_equal, OP.mult)
                mdil = sb.tile([N_M, W], BF16, name="mdil", tag="mdil")
                nc.vector.tensor_scalar(mdil[:], tf_a[:], 128.0, None, OP.is_equal)
                # per-class fg one-hots (value 1.0 where trimap==c+1)
                onehot = []
                for c in range(N_CLASSES):
                    oh = sb.tile([N_OUT, W], BF16, name=f"oh{c}", tag=f"oh{c}")
                    nc.vector.tensor_scalar(oh[:], tf_o[:], float(c + 1), None, OP.is_equal)
                    onehot.append(oh)

                # keep mask: dilate(~consts) then window-any, then valid/9
                d01 = sb.tile([N_PIX, W], BF16, name="d01", tag="d01")
                for (c0, c1) in ((0, CH0), (CH0, W)):
                    dps = psb.tile([N_PIX, CH0], F32, name="bt", tag="bt")
                    dil_box(dps, mdil, c0, c1, wm=band_bf)
                    nc.vector.tensor_scalar(d01[:, c0:c1], dps[0:N_PIX, 0:c1 - c0], 0.0, None, OP.is_gt)
                keep9 = sb.tile([N_WIN, NWC], BF16, name="keep9", tag="keep9")
                keep9f = sb.tile([N_WIN, NWC], F32, name="keep9f", tag="keep9f")
                kps = psf.tile([N_WIN, NWC], F32, name="fwd", tag="fwd")
                fwd_box(kps, d01, wm=band_bf)
                nc.vector.tensor_scalar(keep9[:], kps[:], 0.0, vmask9[:], OP.is_gt, OP.mult)
                nc.vector.tensor_scalar(keep9f[:], kps[:], 0.0, vmask9[:], OP.is_gt, OP.mult)

                # Nkc = 9 * box3T(keep9) + 100 - 100*m  (at output pixel rows)
                nkc = sb.tile([N_OUT, W], BF16, name="nkc", tag="nkc")
                for (c0, c1) in ((0, CH0), (CH0, W)):
                    nps = psb.tile([N_PIX, CH0], F32, name="bt", tag="bt")
                    t_box(nps, keep9, c0, c1, wm=band_bf)
                    nc.vector.tensor_scalar(nkc[:, c0:c1], nps[0:N_OUT, 0:c1 - c0],
                                            NINE_EFF, TRI_CONF, OP.mult, OP.add)
                nc.vector.tensor_sub(nkc[:], nkc[:], m100o[:])

                # ---- color moments (bf16 boxes, fp32 var/adjugate) ----
                imgn_bf = [sb.tile([N_PIX, W], BF16, name=f"imgb{ch}", tag=f"imgb{ch}")
                           for ch in range(3)]
                for ch in range(3):
                    nc.scalar.activation(imgn_bf[ch][:], img_raw[ch][:], COPY,
                                         bias=0.0, scale=1.0 / 255.0)
                # out-aligned (pixel-row) copies for the residual phase
                img_out_raw = [sb.tile([N_OUT, W], F32, name=f"imgor{ch}", tag=f"imgor{ch}")
                               for ch in range(3)]
                imgn_out = [sb.tile([N_OUT, W], BF16, name=f"imgno{ch}", tag=f"imgno{ch}")
                            for ch in range(3)]
                for ch in range(3):
                    nc.sync.dma_start(img_out_raw[ch][:], img_d[s, ch, 2:2 + N_OUT])
                    nc.scalar.activation(imgn_out[ch][:], img_out_raw[ch][:], COPY,
                                         bias=0.0, scale=1.0 / 255.0)

                pairs = [(0, 0), (0, 1), (0, 2), (1, 1), (1, 2), (2, 2)]
                mu_bf = [sb.tile([N_WIN, NWC], BF16, name=f"mub{ch}", tag=f"mub{ch}") for ch in range(3)]
                mneg = [sb.tile([N_WIN, NWC], BF16, name=f"mn{ch}", tag=f"mn{ch}") for ch in range(3)]
                e2 = [sc1.tile([N_WIN, NWC], F32, name=f"e2{i}", tag=f"e2{i}") for i in range(6)]
                for ch in range(3):
                    bps = psf.tile([N_WIN, NWC], F32, name="fwd", tag="fwd")
                    fwd_box(bps, imgn_bf[ch], wm=band_bf)
                    nc.scalar.activation(mu_bf[ch][:], bps[:], COPY, bias=0.0, scale=1.0 / 9.0)
                    nc.vector.tensor_scalar(mneg[ch][:], bps[:], -1.0 / 9.0, None, OP.mult)
                for i, (a, b) in enumerate(pairs):
                    prod = sc1.tile([N_PIX, W], BF16, name="prod", tag="prod")
                    eng = nc.vector if i % 2 == 0 else nc.gpsimd
                    eng.tensor_tensor(prod[:], imgn_bf[a][:], imgn_bf[b][:], OP.mult)
                    bps = psf.tile([N_WIN, NWC], F32, name="fwd", tag="fwd")
                    fwd_box(bps, prod, wm=band_bf)
                    nc.scalar.activation(e2[i][:], bps[:], COPY,
                                         bias=(EPS9 if a == b else 0.0), scale=1.0 / 9.0)

                # var = E2 - mu mu^T (fp32 cancellation), adjugate & det fp32
                var = [sc1.tile([N_WIN, NWC], F32, name=f"var{i}", tag=f"var{i}") for i in range(6)]
                for i, (a, b) in enumerate(pairs):
                    mm = sc1.tile([N_WIN, NWC], F32, name="mm_sc", tag="mm_sc")
                    eng = nc.vector if i % 2 == 0 else nc.gpsimd
                    eng.tensor_tensor(mm[:], mu_bf[a][:], mu_bf[b][:], OP.mult)
                    eng2 = nc.gpsimd if i % 2 == 0 else nc.vector
                    eng2.tensor_tensor(var[i][:], e2[i][:], mm[:], OP.subtract)
                v11, v12, v13, v22, v23, v33 = var

                _fma_n = [0]

                def fma_sub(x1, y1, x2, y2, tag):
                    # returns x1*y1 - x2*y2, alternating engines for balance
                    p1 = sc1.tile([N_WIN, NWC], F32, name="cof_p1", tag="cof_p1")
                    p2 = sc1.tile([N_WIN, NWC], F32, name="cof_p2", tag="cof_p2")
                    o = sc1.tile([N_WIN, NWC], F32, name=tag, tag=tag)
                    k = _fma_n[0]
                    _fma_n[0] += 1
                    e1 = nc.vector if k % 2 == 0 else nc.gpsimd
                    e2_ = nc.gpsimd if k % 2 == 0 else nc.vector
                    e1.tensor_tensor(p1[:], x1[:], y1[:], OP.mult)
                    e2_.tensor_tensor(p2[:], x2[:], y2[:], OP.mult)
                    e1.tensor_tensor(o[:], p1[:], p2[:], OP.subtract)
                    return o

                a11 = fma_sub(v22, v33, v23, v23, "a11")
                a12 = fma_sub(v13, v23, v12, v33, "a12")
                a13 = fma_sub(v12, v23, v13, v22, "a13")
                a22 = fma_sub(v11, v33, v13, v13, "a22")
                a23 = fma_sub(v12, v13, v11, v23, "a23")
                a33 = fma_sub(v11, v22, v12, v12, "a33")
                d1 = sc1.tile([N_WIN, NWC], F32, name="d1", tag="d1")
                d2 = sc1.tile([N_WIN, NWC], F32, name="d2", tag="d2")
                nc.vector.tensor_tensor(d1[:], v11[:], a11[:], OP.mult)
                nc.gpsimd.tensor_tensor(d2[:], v12[:], a12[:], OP.mult)
                nc.vector.tensor_tensor(d1[:], d1[:], d2[:], OP.add)
                nc.gpsimd.tensor_tensor(d2[:], v13[:], a13[:], OP.mult)
                nc.vector.tensor_tensor(d1[:], d1[:], d2[:], OP.add)
                rdet = sc1.tile([N_WIN, NWC], F32, name="rdet", tag="rdet")
                nc.vector.reciprocal(rdet[:], d1[:])
                # keep9-premultiplied inverse: bk = iv' @ t is already keep-masked
                krdet = sc1.tile([N_WIN, NWC], F32, name="krdet", tag="krdet")
                nc.vector.tensor_tensor(krdet[:], rdet[:], keep9f[:], OP.mult)
                iv = [sb.tile([N_WIN, NWC], BF16, name=f"iv{i}", tag=f"iv{i}") for i in range(6)]
                for i, adj in enumerate([a11, a12, a13, a22, a23, a33]):
                    eng = nc.vector if i % 2 == 0 else nc.gpsimd
                    eng.tensor_tensor(iv[i][:], adj[:], krdet[:], OP.mult)
                i11, i12, i13, i22, i23, i33 = iv

                acc_w = sb.tile([N_OUT, 16], F32, name="acc_w", tag="acc_w")

                # ---- per-class ----
                for c in range(N_CLASSES):
                    o = spc.tile([N_PIX, W], F32, name="o", tag="o")
                    nc.sync.dma_start(o[:], cpr_d[s, c])
                    o_bf = spc.tile([N_PIX, W], BF16, name="o_bf", tag="o_bf")
                    nc.scalar.activation(o_bf[:], o[:], COPY, bias=0.0, scale=1.0)
                    o_out_raw = spc.tile([N_OUT, W], F32, name="o_our", tag="o_our")
                    nc.sync.dma_start(o_out_raw[:], cpr_d[s, c, 2:2 + N_OUT])
                    o_out = spc.tile([N_OUT, W], BF16, name="o_out", tag="o_out")
                    nc.gpsimd.tensor_copy(o_out[:], o_out_raw[:])

                    sps = psf.tile([N_WIN, NWC], F32, name="fwd", tag="fwd")
                    fwd_box(sps, o_bf, wm=band_bf)
                    qps = []
                    for ch in range(3):
                        po = sc2.tile([N_PIX, W], BF16, name="po", tag="po")
                        nc.vector.tensor_mul(po[:], imgn_bf[ch][:], o_bf[:])
                        qp = psf.tile([N_WIN, NWC], F32, name="fwd", tag="fwd")
                        fwd_box(qp, po, wm=band_bf)
                        qps.append(qp)

                    # bf16 S/q in SBUF (ScalarE copies off PSUM)
                    s_bf = sc2.tile([N_WIN, NWC], BF16, name="s_bf", tag="s_bf")
                    nc.scalar.activation(s_bf[:], sps[:], COPY, bias=0.0, scale=1.0)
                    q_bf = []
                    for ch in range(3):
                        qb = sc2.tile([N_WIN, NWC], BF16, name=f"qb{ch}", tag=f"qb{ch}")
                        nc.scalar.activation(qb[:], qps[ch][:], COPY, bias=0.0, scale=1.0)
                        q_bf.append(qb)

                    # t_ch = q_ch - mu_ch * S
                    t = []
                    for ch in range(3):
                        ms = sc2.tile([N_WIN, NWC], BF16, name="ms", tag="ms")
                        nc.vector.tensor_mul(ms[:], mu_bf[ch][:], s_bf[:])
                        tt = sc2.tile([N_WIN, NWC], BF16, name=f"t{ch}", tag=f"t{ch}")
                        nc.vector.tensor_sub(tt[:], q_bf[ch][:], ms[:])
                        t.append(tt)

                    # bk = iv' @ t (symmetric, keep-masked); Pool's product runs in
                    # parallel with DVE's pair, DVE joins — no serial ping-pong
                    bk = []
                    for ci, (ia, ib, ic) in enumerate(((i11, i12, i13), (i12, i22, i23), (i13, i23, i33))):
                        vv = sc2.tile([N_WIN, NWC], BF16, name="v_comp", tag="v_comp")
                        p2 = sc2.tile([N_WIN, NWC], BF16, name="v_p2", tag="v_p2")
                        p3 = sc2.tile([N_WIN, NWC], BF16, name="v_p3", tag="v_p3")
                        nc.gpsimd.tensor_tensor(p3[:], ic[:], t[2][:], OP.mult)
                        nc.vector.tensor_mul(vv[:], ia[:], t[0][:])
                        nc.vector.tensor_mul(p2[:], ib[:], t[1][:])
                        nc.vector.tensor_add(vv[:], vv[:], p2[:])
                        nc.vector.tensor_add(vv[:], vv[:], p3[:])
                        bk.append(vv)

                    # A-group fields: kS = keep9*S, mb_ch = (-mu_ch)*bk_ch.
                    # kS/mb2 on Pool: consumed only by the PE t_box, off the DVE spine
                    kS = sc2.tile([N_WIN, NWC], BF16, name="kS", tag="kS")
                    nc.gpsimd.tensor_tensor(kS[:], keep9[:], s_bf[:], OP.mult)
                    mb = []
                    for ch in range(3):
                        mm_ = sc2.tile([N_WIN, NWC], BF16, name=f"mb{ch}", tag=f"mb{ch}")
                        eng = nc.gpsimd if ch >= 1 else nc.vector
                        eng.tensor_tensor(mm_[:], mneg[ch][:], bk[ch][:], OP.mult)
                        mb.append(mm_)

                    # Res = o*Nkc - A' - sum imgn*B; A' = box3T(kS + sum mb) + 100*onehot
                    r = sc2.tile([N_OUT, W], BF16, name="res", tag="res")
                    nc.vector.tensor_mul(r[:], o_out[:], nkc[:])
                    ab = sc2.tile([N_OUT, W], BF16, name="ab", tag="ab")
                    bb = [sc2.tile([N_OUT, W], BF16, name=f"bb{ch}", tag=f"bb{ch}")
                          for ch in range(3)]
                    for (c0, c1) in ((0, CH0), (CH0, W)):
                        cw = c1 - c0
                        aps = psb.tile([N_PIX, CH0], F32, name="bt", tag="bt")
                        t_box(aps, kS, c0, c1, wm=band_bf, start=True, stop=False)
                        for ch in range(3):
                            t_box(aps, mb[ch], c0, c1, wm=band_bf, start=False, stop=False)
                        nc.tensor.matmul(aps[0:N_OUT, 0:cw], ident[:, 0:N_OUT],
                                         onehot[c][0:N_OUT, c0:c1], start=False, stop=True)
                        nc.scalar.activation(ab[:, c0:c1], aps[0:N_OUT, 0:cw], COPY,
                                             bias=0.0, scale=1.0)
                        for ch in range(3):
                            bp = psb.tile([N_PIX, CH0], F32, name="bt", tag="bt")
                            t_box(bp, bk[ch], c0, c1, wm=band_bf)
                            nc.scalar.activation(bb[ch][:, c0:c1], bp[0:N_OUT, 0:cw],
                                                 COPY, bias=0.0, scale=1.0)
                    nc.vector.tensor_sub(r[:], r[:], ab[:])
                    # imgn*B products: ch0 on Pool in parallel, DVE joins the subs
                    p0 = sc2.tile([N_OUT, W], BF16, name="res_p0", tag="res_p0")
                    nc.gpsimd.tensor_tensor(p0[:], imgn_out[0][:], bb[0][:], OP.mult)
                    p = sc2.tile([N_OUT, W], BF16, name="res_p", tag="res_p")
                    for ch in (1, 2):
                        nc.vector.tensor_mul(p[:], imgn_out[ch][:], bb[ch][:])
                        nc.vector.tensor_sub(r[:], r[:], p[:])
                    nc.vector.tensor_sub(r[:], r[:], p0[:])
                    sq = sc2.tile([N_OUT, W], F32, name="sq", tag="sq")
                    nc.scalar.activation(sq[:], r[:], mybir.ActivationFunctionType.Square,
                                         accum_out=acc_w[:, c:c + 1])

                # ---- reduce this sub ----
                accv = sb.tile([N_OUT, 1], F32, name="accv", tag="accv")
                nc.vector.tensor_reduce(accv[:], acc_w[:, 0:N_CLASSES],
                                        axis=mybir.AxisListType.X, op=OP.add)
                if s == 0:
                    nc.vector.tensor_scalar(accm_tot[:], accv[:], rmask[:], None, OP.mult)
                else:
                    accm = sb.tile([N_OUT, 1], F32, name="accm", tag="accm")
                    nc.vector.tensor_scalar(accm[:], accv[:], rmask[:], None, OP.mult)
                    nc.vector.tensor_add(accm_tot[:], accm_tot[:], accm[:])

            fin_ps = psb.tile([1, 1], F32, name="fin", tag="bt")
            nc.tensor.matmul(fin_ps[:], accm_tot[:], ones[:], start=True, stop=True)
            fin = cst.tile([1, 1], F32, name="fin_sb", tag="fin_sb")
            nc.vector.tensor_copy(fin[:], fin_ps[:])
            nc.sync.dma_start(part_d[:], fin[:])

    nc.compile()
    return nc


def _get_program():
    global _PROGRAM
    if _PROGRAM is None:
        _PROGRAM = _build_program()
    return _PROGRAM


def _host_inputs(cprob, img_org, trimap):
    """Slice + pad full inputs into per-core input maps."""
    cprob = np.ascontiguousarray(cprob, dtype=np.float32)
    img_org = np.ascontiguousarray(img_org, dtype=np.float32)
    trimap = np.ascontiguousarray(trimap, dtype=np.int32)

    band = np.zeros((N_M, N_PIX), np.float32)
    for k in range(N_M):
        for m in range(N_PIX):
            if 0 <= k - m <= 2:
                band[k, m] = 1.0
    ones = np.ones((N_OUT, 1), np.float32)
    ident = TRI_CONF * np.eye(N_OUT, dtype=np.float32)

    def rows(arr, lo, hi, fill):
        lead = arr.shape[:-2]
        out = np.full(lead + (hi - lo, arr.shape[-1]), fill, arr.dtype)
        alo, ahi = max(lo, 0), min(hi, H)
        if ahi > alo:
            out[..., alo - lo:ahi - lo, :] = arr[..., alo:ahi, :]
        return out

    in_maps = []
    for core in range(8):
        b = core // 4
        subs = (2 * (core % 4), 2 * (core % 4) + 1)
        cpr = np.stack([rows(cprob[b], SUB_R0[s] - 2, SUB_R0[s] + N_PIX - 2, 0.0)
                        for s in subs])
        img = np.stack([rows(np.moveaxis(img_org[b], -1, 0), SUB_R0[s] - 2,
                             SUB_R0[s] + N_PIX - 2, 0.0) for s in subs])
        trm = np.stack([rows(trimap[b], SUB_R0[s] - 3, SUB_R0[s] + N_M - 3, 0)
                        for s in subs])
        vmask = np.zeros((2, N_WIN, 1), np.float32)
        rmask = np.zeros((2, N_OUT, 1), np.float32)
        for i, s in enumerate(subs):
            r0 = SUB_R0[s]
            for l in range(N_WIN):
                if 0 <= r0 - 2 + l < NWC:
                    vmask[i, l, 0] = 1.0 / 9.0
            own = 65 if s == 7 else 64
            rmask[i, 0:own, 0] = 1.0
        in_maps.append({
            "cpr": cpr, "img": img, "trim": trm,
            "band": band, "ones": ones, "ident": ident,
            "vmask9": vmask, "rmask": rmask,
        })
    return in_maps


def run(cprob, img_org, trimap, trace=False):
    nc = _get_program()
    in_maps = _host_inputs(cprob, img_org, trimap)
    res = run_bass_kernel_spmd(nc, in_maps, list(range(8)), trace=trace)
    total = sum(float(r["partial"][0, 0]) for r in res.results)
    out = np.float32(total / (float(N) * float(N)))
    return out, res


def kernel(cprob, img_org, trimap):
    out, _ = run(cprob, img_org, trimap)
    return out


# revision 20
# speedup vs baseline: 1.0213x; 1.0213x over previous
"""Trainium2 Bass kernel for nn_ClosedFormLoss (closed-form matting Laplacian loss).

Math: per (batch, class), y = keep * (vals @ ow) per 3x3 window followed by a
scatter-add, where vals is the 9x9 matting-Laplacian block built from a
per-window 3x3 color covariance inverse.  With S = box3(o), q_ch =
box3(imgn_ch * o), t = q - mu*S, bk = (keep/9 * inv) @ t:
    Lo(p) = o(p)*Nk(p) - A(p) - sum_ch imgn_ch(p) * B_ch(p)
    A  = box3T(keep/9*S - mu.bk),  B_ch = box3T(bk_ch),  Nk = box3T(keep)
and loss_bc = sum_p (Lo + conf*o - tri*conf)^2 / n^2,  n = H*W.

v2 vs baseline: keep9 is folded into the inverse (iv' = keep9*inv) so the
gpsimd muv chain disappears; the ak assembly (keep9*S - sum mu*bk) happens as a
4-field PSUM accumulation inside the A transpose-box; the per-class target is
injected into the same PSUM group via a 100*I @ onehot_c matmul; the color
moment boxes run in bf16 (1 cyc/row); the residual runs in bf16 off partition
slices of the pixel tiles.  Elementwise work is split DVE/Pool/ACT by measured
cost (DVE TT 0.52 ns/col, Pool TT 1.98, Pool copy 1.39, ACT 0.83).

Sharding: 8 cores = 2 batches x 4 row-quarters; each core processes its
quarter as two 64/65-row sub-stripes and emits one partial sum of squares.
"""

import sys
import numpy as np

sys.path.insert(0, "/opt/trn_rl_repo")

from concourse import bacc, mybir, tile  # noqa: E402
from concourse.bass_utils import run_bass_kernel_spmd  # noqa: E402

F32 = mybir.dt.float32
BF16 = mybir.dt.bfloat16
I32 = mybir.dt.int32
OP = mybir.AluOpType
COPY = mybir.ActivationFunctionType.Copy

N_CLASSES = 7
H = W = 513
NWC = 511          # window grid cols
N = H * W
EPS9 = 1e-7 / 9.0
TRI_CONF = 100.0
import ml_dtypes  # noqa: E402
NINE_EFF = float(1.0 / np.float32(np.float32(1.0 / 9.0).astype(ml_dtypes.bfloat16)))

SUB_R0 = [64 * s for s in range(8)]
N_OUT = 65          # rows computed per sub (65 with 1-row overlap)
N_WIN = 67          # window rows per sub: [r0-2, r0+65)
N_PIX = 69          # pixel rows for box inputs: [r0-2, r0+67)
N_M = 71            # trimap rows for dilate: [r0-3, r0+68)
CH0, CH1 = 257, 256  # column chunks for 513-wide transpose-box outputs

_PROGRAM = None


def _build_program():
    nc = bacc.Bacc("TRN2", target_bir_lowering=False, debug=False, num_devices=8)

    cpr_d = nc.declare_dram_parameter("cpr", [2, N_CLASSES, N_PIX, W], F32, isOutput=False)
    img_d = nc.declare_dram_parameter("img", [2, 3, N_PIX, W], F32, isOutput=False)
    trim_d = nc.declare_dram_parameter("trim", [2, N_M, W], I32, isOutput=False)
    band_d = nc.declare_dram_parameter("band", [N_M, N_PIX], F32, isOutput=False)
    vmask_d = nc.declare_dram_parameter("vmask9", [2, N_WIN, 1], F32, isOutput=False)
    rmask_d = nc.declare_dram_parameter("rmask", [2, N_OUT, 1], F32, isOutput=False)
    ones_d = nc.declare_dram_parameter("ones", [N_OUT, 1], F32, isOutput=False)
    ident_d = nc.declare_dram_parameter("ident", [N_OUT, N_OUT], F32, isOutput=False)
    part_d = nc.declare_dram_parameter("partial", [1, 1], F32, isOutput=True)

    with tile.TileContext(nc) as tc:
        with (
            tc.tile_pool(name="sb", bufs=1) as sb,
            tc.tile_pool(name="spc", bufs=4) as spc,
            tc.tile_pool(name="sc2", bufs=3) as sc2,
            tc.tile_pool(name="sc1", bufs=1) as sc1,
            tc.tile_pool(name="cst", bufs=1) as cst,
            tc.tile_pool(name="psf", bufs=4, space="PSUM") as psf,
            tc.tile_pool(name="psb", bufs=4, space="PSUM") as psb,
        ):
            band = cst.tile([N_M, N_PIX], F32, name="band", tag="band")
            band_bf = cst.tile([N_M, N_PIX], BF16, name="band_bf", tag="band_bf")
            ones = cst.tile([N_OUT, 1], F32, name="ones", tag="ones")
            ident_f = cst.tile([N_OUT, N_OUT], F32, name="ident_f", tag="ident_f")
            ident = cst.tile([N_OUT, N_OUT], BF16, name="ident", tag="ident")
            nc.sync.dma_start(band[:], band_d[:])
            nc.sync.dma_start(ones[:], ones_d[:])
            nc.sync.dma_start(ident_f[:], ident_d[:])
            nc.vector.tensor_copy(band_bf[:], band[:])
            nc.vector.tensor_copy(ident[:], ident_f[:])

            def fwd_box(dst_ps, src, wm, nk=N_PIX, nm=N_WIN):
                # dst[m, j] = sum_{dj} sum_k band[k, m] * src[k, j+dj]
                for dj in range(3):
                    nc.tensor.matmul(
                        dst_ps[0:nm, :], wm[0:nk, 0:nm], src[0:nk, dj:dj + NWC],
                        start=(dj == 0), stop=(dj == 2),
                    )

            def t_box(dst_ps, src, c0, c1, wm, start=True, stop=True):
                # transpose box: out pixel col j <- window cols j-2, j-1, j
                first = start
                for djw in range(3):
                    jlo, jhi = max(c0, djw), min(c1, NWC + djw)
                    nc.tensor.matmul(
                        dst_ps[0:N_OUT, jlo - c0:jhi - c0],
                        wm[0:N_WIN, 0:N_OUT],
                        src[0:N_WIN, jlo - djw:jhi - djw],
                        start=first, stop=(stop and djw == 2),
                    )
                    first = False

            def dil_box(dst_ps, src, c0, c1, wm):
                # 3x3 SAME-pad dilate numerator on the pixel grid
                first = True
                for dj in (-1, 0, 1):
                    jlo, jhi = max(c0, -dj), min(c1, W - dj)
                    nc.tensor.matmul(
                        dst_ps[0:N_PIX, jlo - c0:jhi - c0],
                        wm[0:N_M, 0:N_PIX],
                        src[0:N_M, jlo + dj:jhi + dj],
                        start=first, stop=(dj == 1),
                    )
                    first = False

            accm_tot = cst.tile([N_OUT, 1], F32, name="accm_tot", tag="accm_tot")

            for s in range(2):
                # ---- load per-sub inputs ----
                img_raw = [sb.tile([N_PIX, W], F32, name=f"imgr{ch}", tag=f"imgr{ch}") for ch in range(3)]
                for ch in range(3):
                    nc.sync.dma_start(img_raw[ch][:], img_d[s, ch])
                ti_a = sb.tile([N_M, W], I32, name="ti_a", tag="ti_a")
                nc.sync.dma_start(ti_a[:], trim_d[s])
                vmask9 = sb.tile([N_WIN, 1], F32, name="vmask9", tag="vmask9")
                rmask = sb.tile([N_OUT, 1], F32, name="rmask", tag="rmask")
                nc.sync.dma_start(vmask9[:], vmask_d[s])
                nc.sync.dma_start(rmask[:], rmask_d[s])

                # ---- trimap-derived masks (before moments: keep9 feeds iv') ----
                tf_a = sb.tile([N_M, W], BF16, name="tf_a", tag="tf_a")
                nc.vector.tensor_copy(tf_a[:], ti_a[:])        # 0..128 exact in bf16
                ti_o = sb.tile([N_OUT, W], I32, name="ti_o", tag="ti_o")
                nc.sync.dma_start(ti_o[:], trim_d[s, 3:3 + N_OUT])
                tf_o = sb.tile([N_OUT, W], BF16, name="tf_o", tag="tf_o")
                nc.vector.tensor_copy(tf_o[:], ti_o[:])
                m100o = sb.tile([N_OUT, W], BF16, name="m100o", tag="m100o")
                nc.vector.tensor_scalar(m100o[:], tf_o[:], 128.0, TRI_CONF, OP.is_equal, OP.mult)
                mdil = sb.tile([N_M, W], BF16, name="mdil", tag="mdil")
                nc.vector.tensor_scalar(mdil[:], tf_a[:], 128.0, None, OP.is_equal)
                # per-class fg one-hots (value 1.0 where trimap==c+1)
                onehot = []
                for c in range(N_CLASSES):
                    oh = sb.tile([N_OUT, W], BF16, name=f"oh{c}", tag=f"oh{c}")
                    nc.vector.tensor_scalar(oh[:], tf_o[:], float(c + 1), None, OP.is_equal)
                    onehot.append(oh)

                # keep mask: dilate(~consts) then window-any, then valid/9
                d01 = sb.tile([N_PIX, W], BF16, name="d01", tag="d01")
                for (c0, c1) in ((0, CH0), (CH0, W)):
                    dps = psb.tile([N_PIX, CH0], F32, name="bt", tag="bt")
                    dil_box(dps, mdil, c0, c1, wm=band_bf)
                    nc.vector.tensor_scalar(d01[:, c0:c1], dps[0:N_PIX, 0:c1 - c0], 0.0, None, OP.is_gt)
                keep9 = sb.tile([N_WIN, NWC], BF16, name="keep9", tag="keep9")
                keep9f = sb.tile([N_WIN, NWC], F32, name="keep9f", tag="keep9f")
                kps = psf.tile([N_WIN, NWC], F32, name="fwd", tag="fwd")
                fwd_box(kps, d01, wm=band_bf)
                nc.vector.tensor_scalar(keep9[:], kps[:], 0.0, vmask9[:], OP.is_gt, OP.mult)
                nc.vector.tensor_scalar(keep9f[:], kps[:], 0.0, vmask9[:], OP.is_gt, OP.mult)

                # Nkc = 9 * box3T(keep9) + 100 - 100*m  (at output pixel rows)
                nkc = sb.tile([N_OUT, W], BF16, name="nkc", tag="nkc")
                for (c0, c1) in ((0, CH0), (CH0, W)):
                    nps = psb.tile([N_PIX, CH0], F32, name="bt", tag="bt")
                    t_box(nps, keep9, c0, c1, wm=band_bf)
                    nc.vector.tensor_scalar(nkc[:, c0:c1], nps[0:N_OUT, 0:c1 - c0],
                                            NINE_EFF, TRI_CONF, OP.mult, OP.add)
                nc.vector.tensor_sub(nkc[:], nkc[:], m100o[:])

                # ---- color moments (bf16 boxes, fp32 var/adjugate) ----
                imgn_bf = [sb.tile([N_PIX, W], BF16, name=f"imgb{ch}", tag=f"imgb{ch}")
                           for ch in range(3)]
                for ch in range(3):
                    nc.scalar.activation(imgn_bf[ch][:], img_raw[ch][:], COPY,
                                         bias=0.0, scale=1.0 / 255.0)
                # out-aligned (pixel-row) copies for the residual phase
                img_out_raw = [sb.tile([N_OUT, W], F32, name=f"imgor{ch}", tag=f"imgor{ch}")
                               for ch in range(3)]
                imgn_out = [sb.tile([N_OUT, W], BF16, name=f"imgno{ch}", tag=f"imgno{ch}")
                            for ch in range(3)]
                for ch in range(3):
                    nc.sync.dma_start(img_out_raw[ch][:], img_d[s, ch, 2:2 + N_OUT])
                    nc.scalar.activation(imgn_out[ch][:], img_out_raw[ch][:], COPY,
                                         bias=0.0, scale=1.0 / 255.0)

                pairs = [(0, 0), (0, 1), (0, 2), (1, 1), (1, 2), (2, 2)]
                mu_bf = [sb.tile([N_WIN, NWC], BF16, name=f"mub{ch}", tag=f"mub{ch}") for ch in range(3)]
                mneg = [sb.tile([N_WIN, NWC], BF16, name=f"mn{ch}", tag=f"mn{ch}") for ch in range(3)]
                e2 = [sc1.tile([N_WIN, NWC], F32, name=f"e2{i}", tag=f"e2{i}") for i in range(6)]
                for ch in range(3):
                    bps = psf.tile([N_WIN, NWC], F32, name="fwd", tag="fwd")
                    fwd_box(bps, imgn_bf[ch], wm=band_bf)
                    nc.scalar.activation(mu_bf[ch][:], bps[:], COPY, bias=0.0, scale=1.0 / 9.0)
                    nc.vector.tensor_scalar(mneg[ch][:], bps[:], -1.0 / 9.0, None, OP.mult)
                for i, (a, b) in enumerate(pairs):
                    prod = sc1.tile([N_PIX, W], BF16, name="prod", tag="prod")
                    eng = nc.vector if i % 2 == 0 else nc.gpsimd
                    eng.tensor_tensor(prod[:], imgn_bf[a][:], imgn_bf[b][:], OP.mult)
                    bps = psf.tile([N_WIN, NWC], F32, name="fwd", tag="fwd")
                    fwd_box(bps, prod, wm=band_bf)
                    nc.scalar.activation(e2[i][:], bps[:], COPY,
                                         bias=(EPS9 if a == b else 0.0), scale=1.0 / 9.0)

                # var = E2 - mu mu^T (fp32 cancellation), adjugate & det fp32
                var = [sc1.tile([N_WIN, NWC], F32, name=f"var{i}", tag=f"var{i}") for i in range(6)]
                for i, (a, b) in enumerate(pairs):
                    mm = sc1.tile([N_WIN, NWC], F32, name="mm_sc", tag="mm_sc")
                    eng = nc.vector if i % 2 == 0 else nc.gpsimd
                    eng.tensor_tensor(mm[:], mu_bf[a][:], mu_bf[b][:], OP.mult)
                    eng2 = nc.gpsimd if i % 2 == 0 else nc.vector
                    eng2.tensor_tensor(var[i][:], e2[i][:], mm[:], OP.subtract)
                v11, v12, v13, v22, v23, v33 = var

                _fma_n = [0]

                def fma_sub(x1, y1, x2, y2, tag):
                    # returns x1*y1 - x2*y2, alternating engines for balance
                    p1 = sc1.tile([N_WIN, NWC], F32, name="cof_p1", tag="cof_p1")
                    p2 = sc1.tile([N_WIN, NWC], F32, name="cof_p2", tag="cof_p2")
                    o = sc1.tile([N_WIN, NWC], F32, name=tag, tag=tag)
                    k = _fma_n[0]
                    _fma_n[0] += 1
                    e1 = nc.vector if k % 2 == 0 else nc.gpsimd
                    e2_ = nc.gpsimd if k % 2 == 0 else nc.vector
                    e1.tensor_tensor(p1[:], x1[:], y1[:], OP.mult)
                    e2_.tensor_tensor(p2[:], x2[:], y2[:], OP.mult)
                    e1.tensor_tensor(o[:], p1[:], p2[:], OP.subtract)
                    return o

                a11 = fma_sub(v22, v33, v23, v23, "a11")
                a12 = fma_sub(v13, v23, v12, v33, "a12")
                a13 = fma_sub(v12, v23, v13, v22, "a13")
                a22 = fma_sub(v11, v33, v13, v13, "a22")
                a23 = fma_sub(v12, v13, v11, v23, "a23")
                a33 = fma_sub(v11, v22, v12, v12, "a33")
                d1 = sc1.tile([N_WIN, NWC], F32, name="d1", tag="d1")
                d2 = sc1.tile([N_WIN, NWC], F32, name="d2", tag="d2")
                nc.vector.tensor_tensor(d1[:], v11[:], a11[:], OP.mult)
                nc.gpsimd.tensor_tensor(d2[:], v12[:], a12[:], OP.mult)
                nc.vector.tensor_tensor(d1[:], d1[:], d2[:], OP.add)
                nc.gpsimd.tensor_tensor(d2[:], v13[:], a13[:], OP.mult)
                nc.vector.tensor_tensor(d1[:], d1[:], d2[:], OP.add)
                rdet = sc1.tile([N_WIN, NWC], F32, name="rdet", tag="rdet")
                nc.vector.reciprocal(rdet[:], d1[:])
                # keep9-premultiplied inverse: bk = iv' @ t is already keep-masked
                krdet = sc1.tile([N_WIN, NWC], F32, name="krdet", tag="krdet")
                nc.vector.tensor_tensor(krdet[:], rdet[:], keep9f[:], OP.mult)
                iv = [sb.tile([N_WIN, NWC], BF16, name=f"iv{i}", tag=f"iv{i}") for i in range(6)]
                for i, adj in enumerate([a11, a12, a13, a22, a23, a33]):
                    eng = nc.vector if i % 2 == 0 else nc.gpsimd
                    eng.tensor_tensor(iv[i][:], adj[:], krdet[:], OP.mult)
                i11, i12, i13, i22, i23, i33 = iv

                acc_w = sb.tile([N_OUT, 16], F32, name="acc_w", tag="acc_w")

                # ---- per-class ----
                for c in range(N_CLASSES):
                    o = spc.tile([N_PIX, W], F32, name="o", tag="o")
                    nc.sync.dma_start(o[:], cpr_d[s, c])
                    o_bf = spc.tile([N_PIX, W], BF16, name="o_bf", tag="o_bf")
                    nc.scalar.activation(o_bf[:], o[:], COPY, bias=0.0, scale=1.0)
                    o_out_raw = spc.tile([N_OUT, W], F32, name="o_our", tag="o_our")
                    nc.sync.dma_start(o_out_raw[:], cpr_d[s, c, 2:2 + N_OUT])
                    o_out = spc.tile([N_OUT, W], BF16, name="o_out", tag="o_out")
                    nc.gpsimd.tensor_copy(o_out[:], o_out_raw[:])

                    sps = psf.tile([N_WIN, NWC], F32, name="fwd", tag="fwd")
                    fwd_box(sps, o_bf, wm=band_bf)
                    qps = []
                    for ch in range(3):
                        po = sc2.tile([N_PIX, W], BF16, name="po", tag="po")
                        nc.vector.tensor_mul(po[:], imgn_bf[ch][:], o_bf[:])
                        qp = psf.tile([N_WIN, NWC], F32, name="fwd", tag="fwd")
                        fwd_box(qp, po, wm=band_bf)
                        qps.append(qp)

                    # bf16 S/q in SBUF (ScalarE copies off PSUM)
                    s_bf = sc2.tile([N_WIN, NWC], BF16, name="s_bf", tag="s_bf")
                    nc.scalar.activation(s_bf[:], sps[:], COPY, bias=0.0, scale=1.0)
                    q_bf = []
                    for ch in range(3):
                        qb = sc2.tile([N_WIN, NWC], BF16, name=f"qb{ch}", tag=f"qb{ch}")
                        nc.scalar.activation(qb[:], qps[ch][:], COPY, bias=0.0, scale=1.0)
                        q_bf.append(qb)

                    # t_ch = q_ch - mu_ch * S
                    t = []
                    for ch in range(3):
                        ms = sc2.tile([N_WIN, NWC], BF16, name="ms", tag="ms")
                        nc.vector.tensor_mul(ms[:], mu_bf[ch][:], s_bf[:])
                        tt = sc2.tile([N_WIN, NWC], BF16, name=f"t{ch}", tag=f"t{ch}")
                        nc.vector.tensor_sub(tt[:], q_bf[ch][:], ms[:])
                        t.append(tt)

                    # bk = iv' @ t (symmetric, keep-masked); Pool's product runs in
                    # parallel with DVE's pair, DVE joins — no serial ping-pong
                    bk = []
                    for ci, (ia, ib, ic) in enumerate(((i11, i12, i13), (i12, i22, i23), (i13, i23, i33))):
                        vv = sc2.tile([N_WIN, NWC], BF16, name="v_comp", tag="v_comp")
                        p2 = sc2.tile([N_WIN, NWC], BF16, name="v_p2", tag="v_p2")
                        p3 = sc2.tile([N_WIN, NWC], BF16, name="v_p3", tag="v_p3")
                        nc.gpsimd.tensor_tensor(p3[:], ic[:], t[2][:], OP.mult)
                        nc.vector.tensor_mul(vv[:], ia[:], t[0][:])
                        nc.vector.tensor_mul(p2[:], ib[:], t[1][:])
                        nc.vector.tensor_add(vv[:], vv[:], p2[:])
                        nc.vector.tensor_add(vv[:], vv[:], p3[:])
                        bk.append(vv)

                    # A-group fields: kS = keep9*S, mb_ch = (-mu_ch)*bk_ch.
                    # kS/mb2 on Pool: consumed only by the PE t_box, off the DVE spine
                    kS = sc2.tile([N_WIN, NWC], BF16, name="kS", tag="kS")
                    nc.gpsimd.tensor_tensor(kS[:], keep9[:], s_bf[:], OP.mult)
                    mb = []
                    for ch in range(3):
                        mm_ = sc2.tile([N_WIN, NWC], BF16, name=f"mb{ch}", tag=f"mb{ch}")
                        eng = nc.gpsimd if ch == 2 else nc.vector
                        eng.tensor_tensor(mm_[:], mneg[ch][:], bk[ch][:], OP.mult)
                        mb.append(mm_)

                    # Res = o*Nkc - A' - sum imgn*B; A' = box3T(kS + sum mb) + 100*onehot
                    r = sc2.tile([N_OUT, W], BF16, name="res", tag="res")
                    nc.vector.tensor_mul(r[:], o_out[:], nkc[:])
                    ab = sc2.tile([N_OUT, W], BF16, name="ab", tag="ab")
                    bb = [sc2.tile([N_OUT, W], BF16, name=f"bb{ch}", tag=f"bb{ch}")
                          for ch in range(3)]
                    for (c0, c1) in ((0, CH0), (CH0, W)):
                        cw = c1 - c0
                        aps = psb.tile([N_PIX, CH0], F32, name="bt", tag="bt")
                        t_box(aps, kS, c0, c1, wm=band_bf, start=True, stop=False)
                        for ch in range(3):
                            t_box(aps, mb[ch], c0, c1, wm=band_bf, start=False, stop=False)
                        nc.tensor.matmul(aps[0:N_OUT, 0:cw], ident[:, 0:N_OUT],
                                         onehot[c][0:N_OUT, c0:c1], start=False, stop=True)
                        nc.scalar.activation(ab[:, c0:c1], aps[0:N_OUT, 0:cw], COPY,
                                             bias=0.0, scale=1.0)
                        for ch in range(3):
                            bp = psb.tile([N_PIX, CH0], F32, name="bt", tag="bt")
                            t_box(bp, bk[ch], c0, c1, wm=band_bf)
                            nc.scalar.activation(bb[ch][:, c0:c1], bp[0:N_OUT, 0:cw],
                                                 COPY, bias=0.0, scale=1.0)
                    nc.vector.tensor_sub(r[:], r[:], ab[:])
                    # imgn*B products: ch0 on Pool in parallel, DVE joins the subs
                    p0 = sc2.tile([N_OUT, W], BF16, name="res_p0", tag="res_p0")
                    nc.gpsimd.tensor_tensor(p0[:], imgn_out[0][:], bb[0][:], OP.mult)
                    p = sc2.tile([N_OUT, W], BF16, name="res_p", tag="res_p")
                    for ch in (1, 2):
                        nc.vector.tensor_mul(p[:], imgn_out[ch][:], bb[ch][:])
                        nc.vector.tensor_sub(r[:], r[:], p[:])
                    nc.vector.tensor_sub(r[:], r[:], p0[:])
                    sq = sc2.tile([N_OUT, W], F32, name="sq", tag="sq")
                    nc.scalar.activation(sq[:], r[:], mybir.ActivationFunctionType.Square,
                                         accum_out=acc_w[:, c:c + 1])

                # ---- reduce this sub ----
                accv = sb.tile([N_OUT, 1], F32, name="accv", tag="accv")
                nc.vector.tensor_reduce(accv[:], acc_w[:, 0:N_CLASSES],
                                        axis=mybir.AxisListType.X, op=OP.add)
                if s == 0:
                    nc.vector.tensor_scalar(accm_tot[:], accv[:], rmask[:], None, OP.mult)
                else:
                    accm = sb.tile([N_OUT, 1], F32, name="accm", tag="accm")
                    nc.vector.tensor_scalar(accm[:], accv[:], rmask[:], None, OP.mult)
                    nc.vector.tensor_add(accm_tot[:], accm_tot[:], accm[:])

            fin_ps = psb.tile([1, 1], F32, name="fin", tag="bt")
            nc.tensor.matmul(fin_ps[:], accm_tot[:], ones[:], start=True, stop=True)
            fin = cst.tile([1, 1], F32, name="fin_sb", tag="fin_sb")
            nc.vector.tensor_copy(fin[:], fin_ps[:])
            nc.sync.dma_start(part_d[:], fin[:])

    nc.compile()
    return nc


def _get_program():
    global _PROGRAM
    if _PROGRAM is None:
        _PROGRAM = _build_program()
    return _PROGRAM


def _host_inputs(cprob, img_org, trimap):
    """Slice + pad full inputs into per-core input maps."""
    cprob = np.ascontiguousarray(cprob, dtype=np.float32)
    img_org = np.ascontiguousarray(img_org, dtype=np.float32)
    trimap = np.ascontiguousarray(trimap, dtype=np.int32)

    band = np.zeros((N_M, N_PIX), np.float32)
    for k in range(N_M):
        for m in range(N_PIX):
            if 0 <= k - m <= 2:
                band[k, m] = 1.0
    ones = np.ones((N_OUT, 1), np.float32)
    ident = TRI_CONF * np.eye(N_OUT, dtype=np.float32)

    def rows(arr, lo, hi, fill):
        lead = arr.shape[:-2]
        out = np.full(lead + (hi - lo, arr.shape[-1]), fill, arr.dtype)
        alo, ahi = max(lo, 0), min(hi, H)
        if ahi > alo:
            out[..., alo - lo:ahi - lo, :] = arr[..., alo:ahi, :]
        return out

    in_maps = []
    for core in range(8):
        b = core // 4
        subs = (2 * (core % 4), 2 * (core % 4) + 1)
        cpr = np.stack([rows(cprob[b], SUB_R0[s] - 2, SUB_R0[s] + N_PIX - 2, 0.0)
                        for s in subs])
        img = np.stack([rows(np.moveaxis(img_org[b], -1, 0), SUB_R0[s] - 2,
                             SUB_R0[s] + N_PIX - 2, 0.0) for s in subs])
        trm = np.stack([rows(trimap[b], SUB_R0[s] - 3, SUB_R0[s] + N_M - 3, 0)
                        for s in subs])
        vmask = np.zeros((2, N_WIN, 1), np.float32)
        rmask = np.zeros((2, N_OUT, 1), np.float32)
        for i, s in enumerate(subs):
            r0 = SUB_R0[s]
            for l in range(N_WIN):
                if 0 <= r0 - 2 + l < NWC:
                    vmask[i, l, 0] = 1.0 / 9.0
            own = 65 if s == 7 else 64
            rmask[i, 0:own, 0] = 1.0
        in_maps.append({
            "cpr": cpr, "img": img, "trim": trm,
            "band": band, "ones": ones, "ident": ident,
            "vmask9": vmask, "rmask": rmask,
        })
    return in_maps


def run(cprob, img_org, trimap, trace=False):
    nc = _get_program()
    in_maps = _host_inputs(cprob, img_org, trimap)
    res = run_bass_kernel_spmd(nc, in_maps, list(range(8)), trace=trace)
    total = sum(float(r["partial"][0, 0]) for r in res.results)
    out = np.float32(total / (float(N) * float(N)))
    return out, res


def kernel(cprob, img_org, trimap):
    out, _ = run(cprob, img_org, trimap)
    return out
